# revision 2
# baseline (speedup 1.0000x reference)
"""CON_GATLayer Trainium2 kernel v3: 8-core row-sharded GAT.

v3 = u8 log-score head-pair packing (halves gpsimd scatter passes) + a
union-compacted scatter stream: a single compact pass collects only
referenced sources (with per-source max(kA,kB) copies via prefix
replication); the two half scans then cover only CW=1726 positions
each instead of N+tails=3944. exp is applied after the gather (ACT);
global scale/bias folds into the softmax ratio; edge masking via an
explicit f16 mask multiply.
"""
import math
import numpy as np

import concourse.bass as bass
import concourse.tile as tile
from concourse import bacc, mybir, masks
from concourse.vector_clock import ScopedClock
from concourse.bass_utils import run_bass_kernel_spmd

f32 = mybir.dt.float32
f32r = mybir.dt.float32r
f16 = mybir.dt.float16
u16 = mybir.dt.uint16
u8 = mybir.dt.uint8
i16 = mybir.dt.int16
AF = mybir.ActivationFunctionType
ALU = mybir.AluOpType


class TC(tile.TileContext):
    """TileContext whose final drain splits sem waits into single-wait nops
    (walrus CoreV3 drain codegen rejects >2 wait commands per instruction)."""

    def _drain_and_barrier(self, tick_clock, wait_clock):
        nc = self.nc
        carrier = nc.sync.nop()
        wait_clock.add_sem_waits(
            carrier.ins, ScopedClock({None: tick_clock.global_clock})
        )
        si = carrier.ins.sync_info
        waits = list(si.on_wait) if si and si.on_wait else []
        if len(waits) > 2:
            si.on_wait = []
            for w in waits:
                nop = nc.sync.nop()
                nsi = nop.ins.sync_info
                if nsi is None:
                    nop.ins.sync_info = mybir.SyncInfo(on_wait=[w], on_update=[])
                else:
                    nsi.on_wait = [w]
        nc.sync.drain()
        nc.all_engine_barrier()
        assert self.sems is not None
        popped = nc._tile_sem_poison_stack.pop()
        assert popped is self._sem_poison
        nc.clear_and_free_semaphores(list(self.sems.allocated().values()))
        nc.all_engine_barrier()


# ---------------------------------------------------------------------------
# configuration
# ---------------------------------------------------------------------------

class Cfg:
    def __init__(self, N=3072, IN=256, DH=64, DV=32, H=4, ncores=8):
        P = 128
        self.N, self.IN, self.DH, self.DV, self.H, self.ncores = N, IN, DH, DV, H, ncores
        self.P = P
        assert N % (ncores * P) == 0
        self.RPC = N // ncores          # rows per core
        self.NT = self.RPC // P         # 128-row tiles per core
        self.HALF = N // 2              # scatter dst width
        assert self.HALF * 32 < 2**16, "local_scatter num_elems limit"
        # u8 log-domain quantization of scores: q = floor(att*S + M)
        self.QS = 46.0
        self.QM = 122.5
        # union-compact stream layout (fixed, data-independent margins;
        # host asserts the actual data fits)
        self.R0M = 1376                 # base region width (max referenced)
        self.REP_LENS = [256, 64, 16, 8, 4, 2]
        offs = [self.R0M]
        for L in self.REP_LENS[:-1]:
            offs.append(offs[-1] + L)
        self.REP_OFFS = offs
        self.CW = self.R0M + sum(self.REP_LENS)
        assert self.CW % 2 == 0
        assert self.R0M * 32 < 2**16, "compact dst exceeds gpsimd scratch"
        # matmul chunking
        self.FCH = min(512, N)          # att matmul moving free chunk
        self.PIECE = min(1024, N)       # att psum piece width
        assert N % self.PIECE == 0 and self.PIECE % self.FCH == 0
        self.kchunks = []
        o = 0
        while o < IN:
            c = min(P, IN - o)
            self.kchunks.append((o, c))
            o += c
        self.kchunks.append((IN, 1))    # bias row
        self.INA = IN + 1
        self.VG = DV + 1                # v-group width (v columns + ones)
        self.NJ = N // P                # number of j chunks
        self.HPT = 2 if H >= 2 else 1   # heads per kt/qt tile
        self.NHP = H // self.HPT
        self.NPAIR = H // 2             # u8-packed head pairs


# ---------------------------------------------------------------------------
# host preprocessing
# ---------------------------------------------------------------------------

def build_union_indices(cfg, idxmat, edge):
    """Union-compact scatter schedule for one branch.

    idxmat: (N,N) int source positions (gather G[i,j] = src[i, idxmat[i,j]]).
    edge:   (N,N) bool — only edge positions matter.
    Returns (c_idx (N,N) i16, sA (N,CW) i16, sB (N,CW) i16).

    Layout per row: cs[0:R0M] holds each referenced source once, sorted by
    copies-needed desc; rep region r (offset REP_OFFS[r], len REP_LENS[r])
    is a device-side copy of cs[0:len]. Half-scan pass A reads cs[0:CW]
    with per-position destination columns (rank r of a source sits in rep
    region r-1); same for B.
    """
    N, HALF, CW = cfg.N, cfg.HALF, cfg.CW
    ii, jj = np.nonzero(edge)
    mm = idxmat[ii, jj].astype(np.int64)
    half = (jj >= HALF).astype(np.int64)
    jloc = np.where(half == 1, jj - HALF, jj).astype(np.int64)
    key = (ii.astype(np.int64) * N + mm) * 2 + half
    order = np.argsort(key, kind="stable")
    ii_o, mm_o, half_o, jloc_o, key_o = (
        ii[order], mm[order], half[order], jloc[order], key[order])
    first = np.r_[True, key_o[1:] != key_o[:-1]]
    gs = np.maximum.accumulate(np.where(first, np.arange(len(key_o)), 0))
    rank = np.arange(len(key_o)) - gs           # rank within (i, m, half)
    imkey = ii_o * N + mm_o
    uk, inv = np.unique(imkey, return_inverse=True)
    kA = np.zeros(len(uk), np.int64)
    kB = np.zeros(len(uk), np.int64)
    np.add.at(kA, inv, (half_o == 0))
    np.add.at(kB, inv, (half_o == 1))
    c = np.maximum(kA, kB)                      # copies needed per source
    u_i = uk // N
    u_m = uk % N
    o2 = np.lexsort((-c, u_i))
    ui_s, um_s, c_s = u_i[o2], u_m[o2], c[o2]
    firstr = np.r_[True, ui_s[1:] != ui_s[:-1]]
    gs2 = np.maximum.accumulate(np.where(firstr, np.arange(len(ui_s)), 0))
    base_pos = np.arange(len(ui_s)) - gs2       # base position within row
    R_row = np.bincount(ui_s, minlength=N)
    assert R_row.max() <= cfg.R0M, f"base region overflow: {R_row.max()}"
    cmax = int(c_s.max())
    assert cmax - 1 <= len(cfg.REP_LENS), f"too many copies: {cmax}"
    for r in range(1, cmax):
        cnt = np.bincount(ui_s[c_s >= r + 1], minlength=N)
        assert cnt.max() <= cfg.REP_LENS[r - 1], (
            f"rep region {r} overflow: {cnt.max()} > {cfg.REP_LENS[r-1]}")
    c_idx = np.full((N, N), -1, np.int16)
    c_idx[ui_s, um_s] = base_pos.astype(np.int16)
    bp_of_key = np.full(len(uk), -1, np.int64)
    bp_of_key[o2] = base_pos
    edge_bp = bp_of_key[inv]
    offs_arr = np.array([0] + cfg.REP_OFFS, np.int64)
    pos = np.where(rank == 0, edge_bp,
                   offs_arr[np.minimum(rank, len(offs_arr) - 1)] + edge_bp)
    sA = np.full((N, CW), -1, np.int16)
    sB = np.full((N, CW), -1, np.int16)
    selA = half_o == 0
    sA[ii_o[selA], pos[selA]] = jloc_o[selA].astype(np.int16)
    selB = half_o == 1
    sB[ii_o[selB], pos[selB]] = jloc_o[selB].astype(np.int16)
    return c_idx, sA, sB


def host_prep(cfg, inputs):
    """Returns per-core input maps (list of dicts)."""
    N, IN, DH, DV, H = cfg.N, cfg.IN, cfg.DH, cfg.DV, cfg.H
    x = np.asarray(inputs["x"], np.float32)
    fst = np.asarray(inputs["fst_graph"], np.float32)
    sec = np.asarray(inputs["sec_graph"], np.float32)
    n2c = np.asarray(inputs["n2c"]).astype(np.int32)
    c2n = np.asarray(inputs["c2n"]).astype(np.int32)

    scale = 1.0 / math.sqrt(DH)
    xTa = np.empty((IN + 1, N), np.float32)
    xTa[:IN] = x.T
    xTa[IN] = 1.0

    def aug(W, b, s=1.0):
        Wa = np.empty((IN + 1, W.shape[1]), np.float32)
        Wa[:IN] = np.asarray(W, np.float32) * s
        Wa[IN] = np.asarray(b, np.float32) * s
        return Wa

    # q-side scaled by QS so PSUM att scores arrive pre-scaled for u8 packing
    wq1 = aug(inputs["Wq1"], inputs["bq1"], scale * cfg.QS)
    wk1 = aug(inputs["Wk1"], inputs["bk1"])
    wq2 = aug(inputs["Wq2"], inputs["bq2"], scale * cfg.QS)
    wk2 = aug(inputs["Wk2"], inputs["bk2"])
    # v' layout: per (branch b, head h) group of (DV+1) cols: [Wv_h | ones]
    VG = cfg.VG
    wva = np.zeros((IN + 1, 2 * H * VG), np.float32)
    for b, (Wv, bv) in enumerate(
        [(inputs["Wv1"], inputs["bv1"]), (inputs["Wv2"], inputs["bv2"])]
    ):
        Wv = np.asarray(Wv, np.float32)
        bv = np.asarray(bv, np.float32)
        for h in range(H):
            g = b * H + h
            wva[:IN, g * VG : g * VG + DV] = Wv[:, h * DV : (h + 1) * DV]
            wva[IN, g * VG : g * VG + DV] = bv[h * DV : (h + 1) * DV]
            wva[IN, g * VG + DV] = 1.0

    # branch 1 gathers att2 with c2n, edge1 = fst.T; branch 2 gathers att1
    # with n2c, edge2 = sec.T
    e1 = (fst.T != 0.0)
    e2 = (sec.T != 0.0)
    c1, s1a, s1b = build_union_indices(cfg, c2n, e1)
    c2i, s2a, s2b = build_union_indices(cfg, n2c, e2)
    mk1 = e1.astype(np.float16)
    mk2 = e2.astype(np.float16)

    maps = []
    for c in range(cfg.ncores):
        r0, r1 = c * cfg.RPC, (c + 1) * cfg.RPC
        maps.append(dict(
            xta=xTa, wq1=wq1, wk1=wk1, wq2=wq2, wk2=wk2, wva=wva,
            s1a=np.ascontiguousarray(s1a[r0:r1]),
            s1b=np.ascontiguousarray(s1b[r0:r1]),
            s2a=np.ascontiguousarray(s2a[r0:r1]),
            s2b=np.ascontiguousarray(s2b[r0:r1]),
            c1=np.ascontiguousarray(c1[r0:r1]),
            c2=np.ascontiguousarray(c2i[r0:r1]),
            mk1=np.ascontiguousarray(mk1[r0:r1]),
            mk2=np.ascontiguousarray(mk2[r0:r1]),
        ))
    return maps

# ---------------------------------------------------------------------------
# device kernel builder
# ---------------------------------------------------------------------------

def build_module(cfg, reps=1, skip=()):
    N, IN, DH, DV, H, P = cfg.N, cfg.IN, cfg.DH, cfg.DV, cfg.H, cfg.P
    NT, HALF, CW, R0M = cfg.NT, cfg.HALF, cfg.CW, cfg.R0M
    VG, NJ, INA, RPC = cfg.VG, cfg.NJ, cfg.INA, cfg.RPC
    QS, QM = cfg.QS, cfg.QM
    nc = bacc.Bacc("TRN2", target_bir_lowering=False, debug=False,
                   num_devices=cfg.ncores)

    def dram_in(name, shape, dt):
        return nc.dram_tensor(name, list(shape), dt, kind="ExternalInput").ap()

    xta = dram_in("xta", (INA, N), f32)
    xtq = dram_in("xtq", (INA, RPC), f32)
    wq = [dram_in("wq1", (INA, H * DH), f32), dram_in("wq2", (INA, H * DH), f32)]
    wk = [dram_in("wk1", (INA, H * DH), f32), dram_in("wk2", (INA, H * DH), f32)]
    wva = dram_in("wva", (INA, 2 * H * VG), f32)
    s_in = [[dram_in("s1a", (RPC, CW), i16), dram_in("s1b", (RPC, CW), i16)],
            [dram_in("s2a", (RPC, CW), i16), dram_in("s2b", (RPC, CW), i16)]]
    c_in = [dram_in("c1", (RPC, N), i16), dram_in("c2", (RPC, N), i16)]
    mk_in = [dram_in("mk1", (RPC, N), f16), dram_in("mk2", (RPC, N), f16)]
    y = nc.dram_tensor("y", [RPC, 2 * H * DV], f32, kind="ExternalOutput").ap()

    HPD = cfg.HPT * DH
    with TC(nc) as tc:
        import contextlib
        with contextlib.ExitStack() as ctx:
            const_p = ctx.enter_context(tc.tile_pool(name="const", bufs=1))

            identf = const_p.tile([P, P], f32)
            masks.make_identity(nc, identf[:])
            identh = const_p.tile([P, P], f16)
            nc.vector.tensor_copy(identh[:], identf[:])
            expbias = const_p.tile([P, 1], f32)
            nc.gpsimd.memset(expbias[:], -1.5)
            ubias = const_p.tile([P, 1], f32)
            nc.gpsimd.memset(ubias[:], -QM / QS)

            # persistent projection outputs (fp16)
            kt = [[const_p.tile([HPD, N], f16, tag=f"kt{b}{hp}", name=f"kt{b}{hp}")
                   for hp in range(cfg.NHP)] for b in range(2)]
            qt = [[const_p.tile([HPD, RPC], f16, tag=f"qt{b}{hp}", name=f"qt{b}{hp}")
                   for hp in range(cfg.NHP)] for b in range(2)]
            VW = 2 * H * VG
            v_sb = const_p.tile([P, NJ * VW], f16)

            nkc = len(cfg.kchunks)
            # ---- projection phase (scoped pools, released afterwards) ----
            with tc.tile_pool(name="projsb", bufs=1) as proj_sb, \
                 tc.tile_pool(name="projps", bufs=2, space="PSUM") as proj_ps:
                xt, xq = [], []
                for o, csz in cfg.kchunks:
                    tf = proj_sb.tile([csz, N], f32, tag=f"xs{o}")
                    nc.sync.dma_start(tf[:], xta[o:o + csz, :])
                    tr = proj_sb.tile([csz, N], f32r, tag=f"xt{o}")
                    nc.vector.tensor_copy(tr[:], tf[:])
                    xt.append(tr)
                    tfq = proj_sb.tile([csz, RPC], f32, tag=f"xqs{o}")
                    nc.sync.dma_start(tfq[:], xtq[o:o + csz, :])
                    trq = proj_sb.tile([csz, RPC], f32r, tag=f"xq{o}")
                    nc.vector.tensor_copy(trq[:], tfq[:])
                    xq.append(trq)

                def load_w(ap, width, tag):
                    out = []
                    for o, csz in cfg.kchunks:
                        tf = proj_sb.tile([csz, width], f32, tag=f"{tag}s{o}")
                        nc.sync.dma_start(tf[:], ap[o:o + csz, :])
                        tr = proj_sb.tile([csz, width], f32r, tag=f"{tag}{o}")
                        nc.vector.tensor_copy(tr[:], tf[:])
                        out.append(tr)
                    return out

                wqt = [load_w(wq[b], H * DH, f"wq{b}") for b in range(2)]
                wkt = [load_w(wk[b], H * DH, f"wk{b}") for b in range(2)]
                wvt = load_w(wva, VW, "wv")

                for b in range(2):
                    for hp in range(cfg.NHP):
                        co = hp * HPD
                        for fc in range(0, N, cfg.FCH):
                            ps = proj_ps.tile([HPD, cfg.FCH], f32, tag="pk")
                            for kc in range(nkc):
                                nc.tensor.matmul(
                                    ps[:], wkt[b][kc][:, co:co + HPD],
                                    xt[kc][:, fc:fc + cfg.FCH],
                                    start=(kc == 0), stop=(kc == nkc - 1))
                            nc.scalar.copy(kt[b][hp][:, fc:fc + cfg.FCH], ps[:])
                        for fc in range(0, RPC, cfg.FCH):
                            fw = min(cfg.FCH, RPC - fc)
                            ps = proj_ps.tile([HPD, cfg.FCH], f32, tag="pq")
                            for kc in range(nkc):
                                nc.tensor.matmul(
                                    ps[:, 0:fw], wqt[b][kc][:, co:co + HPD],
                                    xq[kc][:, fc:fc + fw],
                                    start=(kc == 0), stop=(kc == nkc - 1))
                            nc.scalar.copy(qt[b][hp][:, fc:fc + fw], ps[:, 0:fw])
                for jc in range(NJ):
                    ps = proj_ps.tile([P, VW], f32, tag="pv")
                    for kc in range(nkc):
                        nc.tensor.matmul(
                            ps[:], xt[kc][:, jc * P:(jc + 1) * P], wvt[kc][:],
                            start=(kc == 0), stop=(kc == nkc - 1))
                    nc.scalar.copy(v_sb[:, jc * VW:(jc + 1) * VW], ps[:])

            # ---- main pools ----
            att_ps = ctx.enter_context(
                tc.tile_pool(name="att_ps", bufs=2, space="PSUM"))
            tp_ps = ctx.enter_context(
                tc.tile_pool(name="tp_ps", bufs=2, space="PSUM"))
            pv_ps = ctx.enter_context(
                tc.tile_pool(name="pv_ps", bufs=1, space="PSUM"))
            stream_p = ctx.enter_context(tc.tile_pool(name="stream", bufs=1))
            cs_p = ctx.enter_context(tc.tile_pool(name="cs", bufs=1))
            s1_p = ctx.enter_context(tc.tile_pool(name="s1", bufs=1))
            idx_p = ctx.enter_context(tc.tile_pool(name="idx", bufs=1))
            cidx_p = ctx.enter_context(tc.tile_pool(name="cidx", bufs=1))
            mk_p = ctx.enter_context(tc.tile_pool(name="mk", bufs=1))
            g_p = ctx.enter_context(tc.tile_pool(name="gdst", bufs=1))
            G_p = ctx.enter_context(tc.tile_pool(name="G", bufs=2))
            p_p = ctx.enter_context(tc.tile_pool(name="p", bufs=2))
            st_p = ctx.enter_context(tc.tile_pool(name="st", bufs=2))
            out_p = ctx.enter_context(tc.tile_pool(name="out", bufs=2))
            sm_p = ctx.enter_context(tc.tile_pool(name="sm", bufs=4))

            for rep in range(reps):
              for t in range(NT):
                rt0 = t * P
                sidx = [[idx_p.tile([P, CW], i16, tag=f"s{b}{hf}", name=f"sidx{b}{hf}_{t}_{rep}")
                         for hf in range(2)] for b in range(2)]
                cidx = [cidx_p.tile([P, N], i16, tag=f"c{b}", name=f"cidx{b}_{t}_{rep}") for b in range(2)]
                mkt = [mk_p.tile([P, N], f16, tag=f"mk{b}", name=f"mk{b}_{t}_{rep}") for b in range(2)]
                for b in range(2):
                    for hf in range(2):
                        nc.sync.dma_start(sidx[b][hf][:],
                                          s_in[b][hf][rt0:rt0 + P, :])
                    nc.sync.dma_start(cidx[b][:], c_in[b][rt0:rt0 + P, :])
                    nc.sync.dma_start(mkt[b][:], mk_in[b][rt0:rt0 + P, :])

                # ---- pack phase: att matmuls -> f16 exp stream + u8 pair plane
                pks = {}
                s1 = {}
                # emit branch 1 first: its packed planes feed branch 0's
                # gathers which the Pool engine runs first
                for b in (1, 0):
                    for pr in range(cfg.NPAIR):
                        pk = stream_p.tile([P, N], u16, tag=f"pk{b}{pr}",
                                           name=f"pk{b}{pr}_{t}_{rep}")
                        pku8 = pk[:].bitcast(u8)
                        for hh in range(2):
                            h = pr * 2 + hh
                            hp, ho = h // cfg.HPT, (h % cfg.HPT) * DH
                            sh = s1_p.tile([P, N], f16, tag=f"s1{b}{h}",
                                           name=f"s1{b}{h}_{t}_{rep}")
                            if "att" in skip:
                                nc.gpsimd.memset(sh[:, 0:2], 1.0)
                                nc.gpsimd.memset(pk[:, 0:2], 1)
                            else:
                                for po in range(0, N, cfg.PIECE):
                                    ps = att_ps.tile([P, cfg.PIECE], f32, tag="attps")
                                    for fo in range(0, cfg.PIECE, cfg.FCH):
                                        nc.tensor.matmul(
                                            ps[:, fo:fo + cfg.FCH],
                                            qt[b][hp][ho:ho + DH, rt0:rt0 + P],
                                            kt[b][hp][ho:ho + DH,
                                                      po + fo:po + fo + cfg.FCH],
                                            start=True, stop=True)
                                    nc.scalar.activation(
                                        sh[:, po:po + cfg.PIECE], ps[:], AF.Exp,
                                        bias=expbias[:], scale=1.0 / QS)
                                    nc.scalar.activation(
                                        pku8[:, 2 * po + hh:
                                             2 * (po + cfg.PIECE):2],
                                        ps[:], AF.Copy, bias=QM)
                            s1[b, h] = sh
                        pks[b, pr] = pk

                # ---- scatter phase (branch b gathers other branch's plane)
                gdst = {}
                for b in range(2):
                    for pr in range(cfg.NPAIR):
                        src = pks[1 - b, pr]
                        cs = cs_p.tile([P, CW], u16, tag=f"cs{b}{pr}",
                                       name=f"cs{b}{pr}_{t}_{rep}")
                        if "scatter" not in skip:
                            nc.gpsimd.local_scatter(
                                cs[:, 0:R0M], src[:, 0:N], cidx[b][:],
                                channels=P, num_elems=R0M, num_idxs=N)
                            for Lr, off in zip(cfg.REP_LENS, cfg.REP_OFFS):
                                nc.vector.tensor_copy(cs[:, off:off + Lr],
                                                      cs[:, 0:Lr])
                        else:
                            nc.gpsimd.memset(cs[:, 0:2], 1)
                        for hf in range(2):
                            gd = g_p.tile([P, HALF], u16, tag=f"gd{b}{pr}{hf}",
                                          name=f"gd{b}{pr}{hf}_{t}_{rep}")
                            if "scatter" not in skip:
                                nc.gpsimd.local_scatter(
                                    gd[:], cs[:, 0:CW], sidx[b][hf][:],
                                    channels=P, num_elems=HALF, num_idxs=CW)
                            else:
                                nc.gpsimd.memset(gd[:, 0:2], 1)
                            gdst[b, pr, hf] = gd

                # ---- per-head: unpack exp, mask-mul, pv matmul, normalize
                for b in range(2):
                    for h in range(H):
                        pr, hh = h // 2, h % 2
                        pv = pv_ps.tile([VG, P], f32, tag="pv",
                                        name=f"pv{b}_{t}_{h}_{rep}")
                        if "pv" in skip:
                            nc.tensor.matmul(pv[:], v_sb[:, 0:VG], identh[:],
                                             start=True, stop=True)
                        else:
                            sf = p_p.tile([P, N], f16, tag="p",
                                          name=f"sf{b}_{t}_{h}_{rep}")
                            # masked own stream
                            nc.vector.tensor_mul(sf[:], s1[b, h][:], mkt[b][:])
                            for hf in range(2):
                                gu8 = gdst[b, pr, hf][:].bitcast(u8)
                                G = G_p.tile([P, HALF], f16, tag="G",
                                             name=f"G{b}{h}{hf}_{t}_{rep}")
                                nc.scalar.activation(
                                    G[:], gu8[:, hh:2 * HALF:2], AF.Exp,
                                    bias=ubias[:], scale=1.0 / QS)
                                nc.vector.tensor_mul(
                                    sf[:, hf * HALF:(hf + 1) * HALF],
                                    sf[:, hf * HALF:(hf + 1) * HALF], G[:])
                            g_v = b * H + h
                            GRP = 8
                            for jg in range(0, NJ, GRP):
                                gn = min(GRP, NJ - jg)
                                tp = tp_ps.tile([P, GRP * P], f16, tag="tp",
                                                name=f"tp{b}_{t}_{h}_{jg}_{rep}")
                                for q in range(gn):
                                    nc.tensor.transpose(
                                        tp[:, q * P:(q + 1) * P],
                                        sf[:, (jg + q) * P:(jg + q + 1) * P],
                                        identh[:])
                                stt = st_p.tile([P, GRP * P], f16, tag="stt",
                                                name=f"stt{b}_{t}_{h}_{jg}_{rep}")
                                nc.vector.tensor_copy(stt[:, 0:gn * P],
                                                      tp[:, 0:gn * P])
                                for q in range(gn):
                                    jc = jg + q
                                    nc.tensor.matmul(
                                        pv[:], v_sb[:, jc * VW + g_v * VG:
                                                    jc * VW + (g_v + 1) * VG],
                                        stt[:, q * P:(q + 1) * P],
                                        start=(jc == 0), stop=(jc == NJ - 1))
                        pvs = out_p.tile([VG, P], f32, tag="pvs")
                        nc.vector.tensor_copy(pvs[:], pv[:])
                        ot = pv_ps.tile([P, VG], f32, tag="otp")
                        nc.tensor.transpose(ot[:], pvs[:], identf[0:VG, 0:VG])
                        rec = sm_p.tile([P, 1], f32, tag="rec")
                        nc.vector.reciprocal(rec[:], ot[:, DV:DV + 1])
                        res = out_p.tile([P, DV], f32, tag="res")
                        nc.vector.tensor_mul(res[:], ot[:, 0:DV],
                                             rec[:].broadcast_to([P, DV]))
                        nc.sync.dma_start(
                            y[rt0:rt0 + P, (b * H + h) * DV:
                              (b * H + h + 1) * DV], res[:])
    nc.compile()
    return nc


# ---------------------------------------------------------------------------
# entry point
# ---------------------------------------------------------------------------

_CACHE = {}


def _get_module(cfg):
    key = (cfg.N, cfg.IN, cfg.DH, cfg.DV, cfg.H, cfg.ncores)
    if key not in _CACHE:
        _CACHE[key] = build_module(cfg)
    return _CACHE[key]


def kernel(**inputs):
    """Full-input entry point: shards across 8 NeuronCores internally and
    returns the full (N, 2*H*DV) float32 output."""
    cfg = Cfg(N=3072, IN=256, DH=64, DV=32, H=4, ncores=8)
    nc = _get_module(cfg)
    maps = host_prep(cfg, inputs)
    for c, m in enumerate(maps):
        r0 = c * cfg.RPC
        m["xtq"] = np.ascontiguousarray(m["xta"][:, r0:r0 + cfg.RPC])
    res = run_bass_kernel_spmd(nc, maps, list(range(cfg.ncores)), trace=False)
    out = np.concatenate(
        [res.results[c]["y"] for c in range(cfg.ncores)], axis=0)
    return out.astype(np.float32)
